# revision 13
# baseline (speedup 1.0000x reference)
"""GPT-2 attention (B=16, S=1024, E=768, H=12, D=64) on 8 TRN2 NeuronCores.

Sharding: data-parallel over batch - each core processes B_LOC=2 batch
elements with fully replicated weights. No collectives.

v2 design (baseline 390305 ns):
  * x^T / attn-out^T via XBAR dma_start_transpose (fp16) instead of PE
    transposes + DVE copies
  * q/k bias adds on DVE (PSUM->SBUF), weight fp32->fp16 converts on DVE,
    x fp32->fp16 converts on GpSimd: scalar engine runs ONLY the exps
  * exp emitted as fp8e4 (compact banded layout, offsets multiple of 128);
    v in fp8e4 with a fused ones column; attn@v runs fp8 DoubleRow
    (0.5 cyc/row) over kb-pairs via stepped-stride stationary APs.
    Diagonal q-tile (qt==0, low key-count rows) stays fp16 for accuracy.
  * software-pipelined emission: scores(g) interleaved with qkv(g+1)
    chunks, av(g-1) delayed one group so PE never waits on Act exps;
    weights DMA'd on the scalar ring, x/out on the sync ring.
"""

import sys

sys.path.insert(0, "/opt/trn_rl_repo")

from contextlib import ExitStack

import numpy as np

import concourse.bass as bass
import concourse.mybir as mybir
import concourse.tile as tile
from concourse.masks import make_identity

F32 = mybir.dt.float32
F32R = mybir.dt.float32r
F16 = mybir.dt.float16
F8 = mybir.dt.float8e4
AF = mybir.ActivationFunctionType
ALU = mybir.AluOpType
DR = mybir.MatmulPerfMode.DoubleRow

# fp8 DoubleRow av was HW-validated (needs the 2-byte-aligned v8 layout
# below) but is LdWeights-bound at 65-wide moving, so it is net slower;
# XBAR dma_start_transpose corrupts data under load in this kernel.
# Both stay disabled.
USE_DR = False
USE_XBAR = False
DEBUG_DUMP = False
XBAR_ENG = "sync"

B, S, E = 16, 1024, 768
H, D = 12, 64
NCORES = 8
B_LOC = B // NCORES          # 2 batch elements per core
KC = E // 128                # 6 contraction chunks
ST = S // 128                # 8 seq tiles

# band kb covers k in [kb*128,(kb+1)*128), q in [kb*128, S): width 1024-kb*128
BAND_W = [S - kb * 128 for kb in range(ST)]
BAND_OFF = [0, 1024, 1920, 2688, 3328, 3840, 4224, 4480]   # cumulative
EXP_COLS = 4608
OFFU = [o // 128 for o in BAND_OFF]                        # units of 128

# scores/exp per head: 6 sc PSUM tiles.
#   bands: (kb, local offset in the sc tile)
#   acts:  (src_lo, src_hi, exp8 dst offset or None -> exp16 tile)
SC_PLAN = [
    ([(0, 0, True)],                [(0, 128, None), (128, 1024, 128)]),
    ([(1, 0, True)],                [(0, 896, 1024)]),
    ([(2, 0, True)],                [(0, 768, 1920)]),
    ([(3, 0, True)],                [(0, 640, 2688)]),
    ([(4, 0, True), (5, 512, True)], [(0, 896, 3328)]),
    ([(6, 0, False), (7, 256, True)], [(0, 384, 4224)]),
]


def emit(tc, outs, ins):
    nc = tc.nc
    x = ins["hidden_states"]
    wa = ins["W_attn"]
    wp = ins["W_proj"]
    out = outs["out"]
    ba_r = ins["b_attn"].bitcast(F32R)
    bp_r = ins["b_proj"].bitcast(F32R)

    ctx = ExitStack()
    with ctx:
        wpool = ctx.enter_context(tc.tile_pool(name="wpool", bufs=1))
        work = ctx.enter_context(tc.tile_pool(name="work", bufs=1))
        ps = ctx.enter_context(tc.tile_pool(name="ps", bufs=1, space="PSUM"))

        # ---------------- tiny consts (sync ring first) ----------------
        # q/k bias, feature-major [128, 12]: (p, m) = b_attn[m*128 + p]
        ba_qk = wpool.tile([128, 2 * KC], F32)
        nc.sync.dma_start(ba_qk.bitcast(F32R),
                          ba_r[0:2 * E].rearrange("(m p) -> p m", p=128))
        ba_v = wpool.tile([1, E], F32R)
        nc.sync.dma_start(ba_v, ba_r[2 * E:3 * E].unsqueeze(0))
        bp_row = wpool.tile([1, E], F32R)
        nc.sync.dma_start(bp_row, bp_r.unsqueeze(0))

        identity = wpool.tile([128, 128], F32)
        make_identity(nc, identity)
        ones_row32 = wpool.tile([1, 128], F32)
        nc.vector.memset(ones_row32, 1.0)
        ones_row = wpool.tile([1, 128], F32R)
        nc.vector.tensor_copy(ones_row, ones_row32)

        # causal-mask matmul operands: M = UTs.T @ (-250*I)
        uts32 = wpool.tile([128, 128], F32)
        nc.gpsimd.memset(uts32, 1.0)
        nc.gpsimd.affine_select(
            out=uts32, in_=uts32, compare_op=ALU.is_gt,
            fill=0.0, base=0, pattern=[[1, 128]], channel_multiplier=-1,
        )
        uts_h = wpool.tile([128, 128], F16)
        nc.vector.tensor_copy(uts_h, uts32)
        negident_h = wpool.tile([128, 128], F16)
        nc.scalar.activation(negident_h, identity, AF.Copy, scale=-250.0)

        # broadcast v/proj biases to [128, E]
        biasv_bc = wpool.tile([128, E], F32)
        biasp_bc = wpool.tile([128, E], F32)
        for bc_dst, brow in ((biasv_bc, ba_v), (biasp_bc, bp_row)):
            for n0, nw in ((0, 512), (512, 256)):
                bps = ps.tile([128, 512], F32, tag="qk", bufs=2,
                              name=f"bbc{n0}_{brow.name}")
                nc.tensor.matmul(bps[:, 0:nw], ones_row, brow[0:1, n0:n0 + nw],
                                 start=True, stop=True)
                nc.vector.tensor_copy(bc_dst[:, n0:n0 + nw], bps[:, 0:nw])

        # ---------------- persistent weight tiles ----------------
        waq_h = [wpool.tile([128, E], F16, tag=f"waq{k}", name=f"waq{k}")
                 for k in range(KC)]
        wak_h = [wpool.tile([128, E], F16, tag=f"wak{k}", name=f"wak{k}")
                 for k in range(KC)]
        wav_h = [wpool.tile([128, E], F16, tag=f"wav{k}", name=f"wav{k}")
                 for k in range(KC)]
        wp_h = [wpool.tile([128, E], F16, tag=f"wp{k}", name=f"wp{k}")
                for k in range(KC)]

        def stage_w(dst, src_rows, c0, nm):
            stg = work.tile([128, E], F32, tag="wstage", bufs=2, name=nm)
            nc.scalar.dma_start(stg, src_rows[:, c0:c0 + E])
            nc.vector.tensor_copy(dst, stg)

        # ---------------- per-batch state ----------------
        # xT[b][p, st, c, s] = x16_st[s, c*128 + p]   (fp16)
        xT = [work.tile([128, ST, KC, 128], F16, tag="xT", bufs=2,
                        name=f"xT{b}") for b in range(B_LOC)]
        v8 = [work.tile([128, ST, H, D + 2], F8, tag="v8", bufs=1,
                        name=f"v8_{b}") for b in range(B_LOC)]
        v16 = [work.tile([128, H, D + 1], F16, tag="v16", bufs=1,
                         name=f"v16_{b}") for b in range(B_LOC)]
        ao = [[work.tile([128, E], F16, tag=f"ao{st}", bufs=1,
                         name=f"ao{st}_{b}") for st in range(ST)]
              for b in range(B_LOC)]

        ident_h = wpool.tile([128, 128], F16)
        nc.vector.tensor_copy(ident_h, identity)

        x16s = {}

        def x_load(b, sts):
            xrb = x.bitcast(F32R)
            for st in sts:
                xin = work.tile([128, E], F32R, tag="xin", bufs=2,
                                name=f"xin{b}_{st}")
                nc.sync.dma_start(xin, xrb[b, st * 128:(st + 1) * 128, :])
                x16 = work.tile([128, E], F16, tag="x16", bufs=10,
                                name=f"x16_{b}_{st}")
                nc.gpsimd.tensor_copy(x16, xin.bitcast(F32))
                x16s[(b, st)] = x16

        def x_transpose(b, sts):
            xbar_eng = getattr(nc, XBAR_ENG)
            for st in sts:
                x16 = x16s[(b, st)]
                if USE_XBAR:
                    xbar_eng.dma_start_transpose(xT[b][:, st], x16)
                else:
                    for k in range(KC):
                        tp = ps.tile([128, 128], F16, tag="av", bufs=2,
                                     name=f"xtr{b}_{st}_{k}")
                        nc.tensor.transpose(
                            tp, x16[:, k * 128:(k + 1) * 128], ident_h)
                        nc.vector.tensor_copy(xT[b][:, st, k], tp)

        def x_phase(b):
            x_load(b, range(ST))
            x_transpose(b, range(ST))

        def v_phase(b):
            for st in range(ST):
                for n0, nw in ((0, 512), (512, 256)):
                    acc = ps.tile([128, 512], F32, tag="qk", bufs=2,
                                  name=f"vacc{b}_{st}_{n0}")
                    for k in range(KC):
                        nc.tensor.matmul(
                            acc[:, 0:nw], xT[b][:, st, k],
                            wav_h[k][:, n0:n0 + nw],
                            start=(k == 0), stop=(k == KC - 1))
                    h0 = n0 // D
                    nh = nw // D
                    nc.vector.tensor_add(
                        v8[b][:, st, h0:h0 + nh, 0:D],
                        acc[:, 0:nw].rearrange("p (h d) -> p h d", d=D),
                        biasv_bc[:, n0:n0 + nw].rearrange(
                            "p (h d) -> p h d", d=D))
                    if st == 0:
                        nc.vector.tensor_add(
                            v16[b][:, h0:h0 + nh, 0:D],
                            acc[:, 0:nw].rearrange("p (h d) -> p h d", d=D),
                            biasv_bc[:, n0:n0 + nw].rearrange(
                                "p (h d) -> p h d", d=D))
                nc.vector.memset(v8[b][:, st, :, D:D + 1], 1.0)
            nc.vector.memset(v16[b][:, :, D:D + 1], 1.0)

        def qk_dst(b, g, di):
            """Emit one of the 4 q/k dsts of group g: di -> (pair tt, q|k)."""
            tt, is_k = di // 2, di % 2
            t = 2 * g + tt
            m = (KC + t) if is_k else t
            wqk = wak_h if is_k else waq_h
            tag = "kt" if is_k else "qt"
            dst = work.tile([128, S], F16, tag=tag, bufs=4,
                            name=f"{tag}{t}_{b}")
            for c0 in (0, 512):
                qkps = ps.tile([128, 512], F32, tag="qk", bufs=2,
                               name=f"qk{b}_{m}_{c0}")
                st0 = c0 // 128
                for k in range(KC):
                    nc.tensor.matmul(
                        qkps[:, 0:512], wqk[k][:, t * 128:(t + 1) * 128],
                        xT[b][:, st0:st0 + 4, k, :],
                        start=(k == 0), stop=(k == KC - 1))
                nc.vector.tensor_add(
                    dst[:, c0:c0 + 512], qkps[:, 0:512],
                    ba_qk[:, m:m + 1].broadcast_to((128, 512)))
            return dst

        def sc_head(b, g, hh, q_r, k_r, exp8, exp16):
            po = (hh % 2) * 64
            for bands, acts in SC_PLAN:
                sc = ps.tile([128, 1024], F32, tag="sc", bufs=2,
                             name=f"sc{b}_{g}_{hh}_{bands[0][0]}")
                for kb, lo, mstop in bands:
                    k0 = kb * 128
                    c0 = k0
                    while c0 < S:
                        cw = min(512, S - c0)
                        ll = lo + (c0 - k0)
                        first = c0 == k0
                        nc.tensor.matmul(
                            sc[:, ll:ll + cw],
                            k_r[po:po + 64, k0:k0 + 128],
                            q_r[po:po + 64, c0:c0 + cw],
                            start=(ll % 512 == 0),
                            stop=(mstop if first else True))
                        c0 += cw
                for lo, hi_, doff in acts:
                    if doff is None:
                        nc.scalar.activation(exp16, sc[:, lo:hi_],
                                             AF.Exp, scale=0.125)
                    else:
                        nc.scalar.activation(
                            exp8[:, doff:doff + (hi_ - lo)],
                            sc[:, lo:hi_], AF.Exp, scale=0.125)
                for kb, lo, mstop in bands:
                    if kb == 0:
                        dblk = exp16
                    else:
                        dblk = exp8[:, BAND_OFF[kb]:BAND_OFF[kb] + 128]
                    # causal mask: keep exp[r, c] only where c >= r
                    nc.gpsimd.affine_select(
                        out=dblk, in_=dblk, compare_op=ALU.is_ge,
                        fill=0.0, base=0, pattern=[[1, 128]],
                        channel_multiplier=-1)

        def av_group(b, g, exps8, exps16):
            for qt in range(ST):
                av4 = ps.tile([128, 4, D + 1], F32, tag="av", bufs=2,
                              name=f"av{b}_{g}_{qt}")
                ops = []          # (hi, kind, kb)
                for hi in range(4):
                    if qt == 0:
                        ops.append((hi, "f16", 0))
                        continue
                    kb = 0
                    while kb + 1 <= qt:
                        ops.append((hi, "pair", kb))
                        kb += 2
                    if kb == qt:
                        ops.append((hi, "single", kb))
                for i, (hi, kind, kb) in enumerate(ops):
                    h = 4 * g + hi
                    st_ = (i == 0)
                    sp_ = (i == len(ops) - 1)
                    if kind == "f16":
                        nc.tensor.matmul(av4[:, hi, :], exps16[hi],
                                         v16[b][:, h, :],
                                         start=st_, stop=sp_)
                    elif kind == "single":
                        expv = exps8[hi].rearrange("p (a c) -> p a c", c=128)
                        nc.tensor.matmul(
                            av4[:, hi, :],
                            expv[:, OFFU[kb] + (qt - kb), :],
                            v8[b][:, kb, h, 0:D + 1],
                            start=st_, stop=sp_)
                    elif USE_DR:
                        sg = OFFU[kb + 1] - OFFU[kb] - 1
                        u = OFFU[kb] + (qt - kb)
                        expv = exps8[hi].rearrange("p (a c) -> p a c", c=128)
                        nc.tensor.matmul(
                            av4[:, hi, :],
                            expv[:, u:u + 2 * sg:sg, :],
                            v8[b][:, kb:kb + 2, h, 0:D + 1],
                            start=st_, stop=sp_, perf_mode=DR)
                    else:
                        expv = exps8[hi].rearrange("p (a c) -> p a c", c=128)
                        nc.tensor.matmul(
                            av4[:, hi, :], expv[:, OFFU[kb] + (qt - kb), :],
                            v8[b][:, kb, h, 0:D + 1], start=st_, stop=False)
                        nc.tensor.matmul(
                            av4[:, hi, :],
                            expv[:, OFFU[kb + 1] + (qt - kb - 1), :],
                            v8[b][:, kb + 1, h, 0:D + 1], start=False, stop=sp_)
                rc4 = work.tile([128, 4, 1], F32, tag="rc", bufs=4,
                                name=f"rc{b}_{g}_{qt}")
                nc.vector.reciprocal(rc4, av4[:, :, D:D + 1])
                nc.vector.tensor_mul(
                    ao[b][qt][:, g * 256:(g + 1) * 256].rearrange(
                        "p (h d) -> p h d", d=D),
                    av4[:, :, 0:D],
                    rc4.broadcast_to((128, 4, D)))

        def proj_phase(b):
            for st in range(ST):
                aoTt = work.tile([128, KC, 128], F16, tag="aoT", bufs=2,
                                 name=f"aoT{b}_{st}")
                if USE_XBAR:
                    getattr(nc, XBAR_ENG).dma_start_transpose(aoTt, ao[b][st])
                else:
                    for k in range(KC):
                        tp = ps.tile([128, 128], F16, tag="av", bufs=2,
                                     name=f"aotr{b}_{st}_{k}")
                        nc.tensor.transpose(
                            tp, ao[b][st][:, k * 128:(k + 1) * 128], ident_h)
                        nc.vector.tensor_copy(aoTt[:, k], tp)
                outt = work.tile([128, E], F32, tag="outt", bufs=2,
                                 name=f"outt{b}_{st}")
                for n0, nw in ((0, 512), (512, 256)):
                    pacc = ps.tile([128, 512], F32, tag="qk", bufs=2,
                                   name=f"pacc{b}_{st}_{n0}")
                    for k in range(KC):
                        nc.tensor.matmul(
                            pacc[:, 0:nw], aoTt[:, k],
                            wp_h[k][:, n0:n0 + nw],
                            start=(k == 0), stop=(k == KC - 1))
                    nc.vector.tensor_add(outt[:, n0:n0 + nw], pacc[:, 0:nw],
                                         biasp_bc[:, n0:n0 + nw])
                nc.scalar.dma_start(out[b, st * 128:(st + 1) * 128, :], outt)

        # ---------------- emission schedule ----------------
        x_phase(0)
        # weights on the scalar ring: v columns first, then q, k, proj
        for k in range(KC):
            stage_w(wav_h[k], wa[k * 128:(k + 1) * 128, :], 2 * E, f"wsv{k}")
        for k in range(KC):
            stage_w(waq_h[k], wa[k * 128:(k + 1) * 128, :], 0, f"wsq{k}")
        for k in range(KC):
            stage_w(wak_h[k], wa[k * 128:(k + 1) * 128, :], E, f"wsk{k}")
        for k in range(KC):
            stage_w(wp_h[k], wp[k * 128:(k + 1) * 128, :], 0, f"wsp{k}")

        v_phase(0)
        x_load(1, range(ST))

        exp8_tiles = {}
        exp16_tiles = {}

        def exps_for(b, g):
            key = (b, g)
            if key not in exp8_tiles:
                exp8_tiles[key] = [
                    work.tile([128, EXP_COLS], F8, tag="exp", bufs=8,
                              name=f"exp{b}_{g}_{hi}") for hi in range(4)]
                exp16_tiles[key] = [
                    work.tile([128, 128], F16, tag="e16", bufs=8,
                              name=f"e16_{b}_{g}_{hi}") for hi in range(4)]
            return exp8_tiles[key], exp16_tiles[key]

        qk_tiles = {}

        def qk_group(b, g, di):
            key = (b, g)
            if key not in qk_tiles:
                qk_tiles[key] = [None] * 4
            qk_tiles[key][di] = qk_dst(b, g, di)

        # prime group (0,0)
        for di in range(4):
            qk_group(0, 0, di)
        if DEBUG_DUMP:
            nc.sync.dma_start(outs["d_xT"], xT[0][:])
            nc.sync.dma_start(outs["d_v8"], v8[0][:])
            nc.sync.dma_start(outs["d_v16"], v16[0][:])
            nc.sync.dma_start(outs["d_q"], qk_tiles[(0, 0)][0][:])
            nc.sync.dma_start(outs["d_k"], qk_tiles[(0, 0)][1][:])

        steps = [(b, g) for b in range(B_LOC) for g in range(3)]
        for si, (b, g) in enumerate(steps):
            nxt = steps[si + 1] if si + 1 < len(steps) else None
            e8, e16 = exps_for(b, g)
            qks = qk_tiles[(b, g)]
            for hh in range(4):
                tt = hh // 2
                sc_head(b, g, hh, qks[2 * tt], qks[2 * tt + 1],
                        e8[hh], e16[hh])
                if DEBUG_DUMP and si == 0 and hh == 0:
                    nc.sync.dma_start(outs["d_e8"], e8[0][:])
                    nc.sync.dma_start(outs["d_e16"], e16[0][:])
                if nxt is not None:
                    qk_group(nxt[0], nxt[1], hh)
                if si == 0 and hh in (1, 2):
                    x_transpose(1, range(4 * (hh - 1), 4 * hh))
            if g >= 1:
                av_group(b, g - 1, *exps_for(b, g - 1))
            if g == 2:
                av_group(b, 2, e8, e16)
                if DEBUG_DUMP and b == 0:
                    nc.sync.dma_start(outs["d_ao0"], ao[0][0][:])
                if b == 0:
                    v_phase(1)
                proj_phase(b)


def build():
    from concourse import bacc

    nc = bacc.Bacc("TRN2", target_bir_lowering=False, debug=False)
    ins = {
        "hidden_states": nc.dram_tensor(
            "hidden_states", [B_LOC, S, E], F32, kind="ExternalInput").ap(),
        "W_attn": nc.dram_tensor("W_attn", [E, 3 * E], F32,
                                 kind="ExternalInput").ap(),
        "b_attn": nc.dram_tensor("b_attn", [3 * E], F32,
                                 kind="ExternalInput").ap(),
        "W_proj": nc.dram_tensor("W_proj", [E, E], F32,
                                 kind="ExternalInput").ap(),
        "b_proj": nc.dram_tensor("b_proj", [E], F32, kind="ExternalInput").ap(),
    }
    outs = {
        "out": nc.dram_tensor("out", [B_LOC, S, E], F32,
                              kind="ExternalOutput").ap(),
    }
    if DEBUG_DUMP:
        for nm, shp, dt_ in (
                ("d_xT", [128, ST, KC, 128], F16),
                ("d_q", [128, S], F16), ("d_k", [128, S], F16),
                ("d_e8", [128, EXP_COLS], F8), ("d_e16", [128, 128], F16),
                ("d_v8", [128, ST, H, D + 2], F8),
                ("d_v16", [128, H, D + 1], F16),
                ("d_ao0", [128, E], F16)):
            outs[nm] = nc.dram_tensor(nm, shp, dt_,
                                      kind="ExternalOutput").ap()
    with tile.TileContext(nc) as tc:
        emit(tc, outs, ins)
    nc.compile()
    return nc


_CACHED_NC = None


def kernel(hidden_states, W_attn, b_attn, W_proj, b_proj, trace=False):
    global _CACHED_NC
    from concourse.bass_utils import run_bass_kernel_spmd

    if _CACHED_NC is None:
        _CACHED_NC = build()
    nc = _CACHED_NC

    hidden_states = np.ascontiguousarray(hidden_states, dtype=np.float32)
    W_attn = np.ascontiguousarray(W_attn, dtype=np.float32)
    b_attn = np.ascontiguousarray(b_attn, dtype=np.float32)
    W_proj = np.ascontiguousarray(W_proj, dtype=np.float32)
    b_proj = np.ascontiguousarray(b_proj, dtype=np.float32)

    in_maps = []
    for c in range(NCORES):
        in_maps.append({
            "hidden_states": hidden_states[c * B_LOC:(c + 1) * B_LOC],
            "W_attn": W_attn, "b_attn": b_attn,
            "W_proj": W_proj, "b_proj": b_proj,
        })
    res = run_bass_kernel_spmd(nc, in_maps, core_ids=list(range(NCORES)),
                               trace=trace)
    out = np.concatenate([res.results[c]["out"] for c in range(NCORES)], axis=0)
    kernel.last_result = res
    return out
